# revision 10
# baseline (speedup 1.0000x reference)
"""Relative-position attention (TransformerXL-style) on 8 TRN2 NeuronCores.

Sharding: data-parallel over batch (b=8 -> 1 batch element per core); weights
replicated. No collectives needed.

Per-core pipeline (n=1024, dim=512, heads=8, d_head=64), head-PAIR batched:
  qT = Wq^T x^T, kT = Wk^T x^T   [inner, n]   (bf16 matmuls, fp32 psum)
  v2 = x Wv scattered into [n, 8*(64+1)] with a ones column per head
       (the ones column makes the AV matmul also produce z = row-sum of P)
  per (128-row query tile m, head pair hp = heads 2hp,2hp+1):
    T_psum[128, w] = qh^T relT[:, band]  per head (2 matmuls)
    t_pair (fp8) <- T_psum + clip-tail fills  [128, 2*1152]
    pos8z via ONE diagonal-skew SBUF->SBUF DMA into [pos_lo|pos_hi|zeros]
        per-head layout (zeros pre-memset once in 2 parity buffers)
    s_pair[128, 2048] = qh^T kTh  (4 matmuls, 2 per head)
    s_pair += pos  (2 fp8 DoubleRow matmuls per head: [I|I] stationary with
        the zeros block as second k-tile -> 0.5 cycles/row)
    p_pair (bf16) = exp(0.125 * s_pair)  (ONE batched ACT op, no accum)
    ptT[128, 16, 128] <- p_pair via ONE DMA-transpose (16x128 xbar tiles)
    av[128, 65] = sum_jb ptT-block-stationary @ v2_jb_h  (8 matmuls, N=65:
        col 64 accumulates z since v2 col 64 is all-ones)
    r = 1/av[:, 64]; o_att[m][:, head] = av[:, :64] * r  (scaled evacuation)
  o_att[m] -> toT via DMA-xbar transpose (pipelined one m deep)
  out_m[128, 512] = toT-blocks^T Wo + ones^T bo  (5 matmuls, K=1 bias trick)

The rel-pos table is host-preprocessed into relT[d, c] = rel_emb[1024 -
clip(c - 511, 0, 1024), d] so that pos_attn[i, j] = (q_i . relT[:, j - i +
1023]) and clipping is baked into the padded table.
"""
import sys

sys.path.insert(0, "/opt/trn_rl_repo")

import numpy as np

import concourse.bass as bass
import concourse.bacc as bacc
import concourse.mybir as mybir
import concourse.tile as tile
from concourse.ap import AP
from concourse.bass_utils import run_bass_kernel_spmd

F32 = mybir.dt.float32
BF16 = mybir.dt.bfloat16
FP8 = mybir.dt.float8e4
DR = mybir.MatmulPerfMode.DoubleRow

B, N, DIM = 8, 1024, 512
HEADS, DH = 8, 64
INNER = HEADS * DH
MAX_POS = 512
RELW = 2 * MAX_POS + 1        # 1025 rel-emb rows
RELTW = 2047                  # extended/clip-padded table width
TW = 1151                     # per-query-tile T width (1024 + 127)
TWPAD = 1152
PZW = 1536                    # per-head pos8z width: 512 lo + 512 hi + 512 zero
KC = DIM // 128               # 4 contraction chunks
MT = N // 128                 # 8 query row tiles
VW = DH + 1                   # per-head v2 width (ones col at 64)
SCALE = DH ** -0.5

_CACHE = {}


def _build_nc():
    nc = bacc.Bacc()
    xT_in = nc.declare_dram_parameter("xT", [DIM, N], BF16, isOutput=False)
    wq_in = nc.declare_dram_parameter("wq", [DIM, INNER], BF16, isOutput=False)
    wk_in = nc.declare_dram_parameter("wk", [DIM, INNER], BF16, isOutput=False)
    wv_in = nc.declare_dram_parameter("wv", [DIM, INNER], BF16, isOutput=False)
    wo_in = nc.declare_dram_parameter("wo", [INNER, DIM], BF16, isOutput=False)
    rel_in = nc.declare_dram_parameter("relT", [128, RELTW], BF16, isOutput=False)
    id2_in = nc.declare_dram_parameter("id2", [128, 256], FP8, isOutput=False)
    bo_in = nc.declare_dram_parameter("bo", [1, DIM], BF16, isOutput=False)
    out_ext = nc.declare_dram_parameter("out", [N, DIM], F32, isOutput=True)

    with tile.TileContext(nc) as tc:
        with tc.tile_pool(name="persist", bufs=1) as pp:
            # ---- load persistent operands ----
            xT_sb = [pp.tile([128, N], BF16, name=f"xT{k}") for k in range(KC)]
            wq_sb = [pp.tile([128, INNER], BF16, name=f"wq{k}") for k in range(KC)]
            wk_sb = [pp.tile([128, INNER], BF16, name=f"wk{k}") for k in range(KC)]
            wv_sb = [pp.tile([128, INNER], BF16, name=f"wv{k}") for k in range(KC)]
            wo_sb = [pp.tile([128, DIM], BF16, name=f"wo{k}") for k in range(KC)]
            rel_sb = pp.tile([128, RELTW], BF16)
            id2_sb = pp.tile([128, 256], FP8)
            bo_sb = pp.tile([1, DIM], BF16)
            ones_sb = pp.tile([1, 128], BF16)
            onesw_sb = pp.tile([128, 512], FP8)
            for k in range(KC):
                nc.sync.dma_start(out=xT_sb[k][:], in_=xT_in[128 * k:128 * (k + 1), :])
                nc.sync.dma_start(out=wq_sb[k][:], in_=wq_in[128 * k:128 * (k + 1), :])
                nc.sync.dma_start(out=wk_sb[k][:], in_=wk_in[128 * k:128 * (k + 1), :])
            for k in range(KC):
                nc.sync.dma_start(out=wv_sb[k][:], in_=wv_in[128 * k:128 * (k + 1), :])
            nc.sync.dma_start(out=rel_sb[:], in_=rel_in[:])
            nc.sync.dma_start(out=id2_sb[:], in_=id2_in[:])
            for k in range(KC):
                nc.sync.dma_start(out=wo_sb[k][:], in_=wo_in[128 * k:128 * (k + 1), :])
            nc.sync.dma_start(out=bo_sb[:], in_=bo_in[:])
            nc.gpsimd.memset(ones_sb[:], 1.0)
            nc.gpsimd.memset(onesw_sb[:], 1.0)

            # pos8z parity buffers: [pos_lo | pos_hi | zeros] per head; the
            # zeros block doubles as the second DoubleRow k-tile.
            pos8z = [pp.tile([128, 2 * PZW], FP8, name=f"pos8z{i}")
                     for i in range(6)]
            for i in range(len(pos8z)):
                z_ap = AP(pos8z[i].tensor, pos8z[i].offset + 1024,
                          [[2 * PZW, 128], [PZW, 2], [1, 512]])
                nc.gpsimd.memset(z_ap, 0.0)

            # ---- projections ----
            qT_sb = [pp.tile([128, N], BF16, name=f"qT{t}") for t in range(KC)]
            kT_sb = [pp.tile([128, N], BF16, name=f"kT{t}") for t in range(KC)]
            v2_sb = [pp.tile([128, HEADS * VW], BF16, name=f"v{t}")
                     for t in range(MT)]
            o_att = [pp.tile([128, INNER], BF16, name=f"oatt{t}") for t in range(MT)]
            for t in range(MT):  # ones columns for z accumulation
                ones_ap = AP(v2_sb[t].tensor, v2_sb[t].offset + DH,
                             [[HEADS * VW, 128], [VW, HEADS], [1, 1]])
                nc.gpsimd.memset(ones_ap, 1.0)

            with tc.tile_pool(name="proj_ps", bufs=4, space="PSUM") as proj_ps:
                for t in range(KC):          # qT / kT tiles: inner rows 128t..
                    for jc in range(2):      # n column chunks of 512
                        for which, w_sb, dst in (("q", wq_sb, qT_sb), ("k", wk_sb, kT_sb)):
                            ps = proj_ps.tile([128, 512], F32, tag="pps",
                                              name=f"ps{which}{t}{jc}")
                            for k in range(KC):
                                nc.tensor.matmul(
                                    ps[:],
                                    w_sb[k][:, 128 * t:128 * (t + 1)],
                                    xT_sb[k][:, 512 * jc:512 * (jc + 1)],
                                    start=(k == 0), stop=(k == KC - 1))
                            if (t + jc) % 2 == 0:
                                nc.scalar.copy(dst[t][:, 512 * jc:512 * (jc + 1)], ps[:])
                            else:
                                nc.vector.tensor_copy(dst[t][:, 512 * jc:512 * (jc + 1)], ps[:])
                for t in range(MT):          # v tiles: n rows 128t..
                    ps = proj_ps.tile([128, 512], F32, tag="pps", name=f"psv{t}")
                    for k in range(KC):
                        nc.tensor.matmul(
                            ps[:],
                            xT_sb[k][:, 128 * t:128 * (t + 1)],
                            wv_sb[k][:],
                            start=(k == 0), stop=(k == KC - 1))
                    v_out = AP(v2_sb[t].tensor, v2_sb[t].offset,
                               [[HEADS * VW, 128], [VW, HEADS], [1, DH]])
                    v_in = AP(ps.tensor, ps.offset,
                              [[512, 128], [DH, HEADS], [1, DH]])
                    if t % 2 == 0:
                        nc.scalar.copy(v_out, v_in)
                    else:
                        nc.vector.tensor_copy(v_out, v_in)

            # ---- attention ----
            with tc.tile_pool(name="attn_sb", bufs=3) as asb, \
                 tc.tile_pool(name="s_ps", bufs=2, space="PSUM") as sps, \
                 tc.tile_pool(name="t_ps", bufs=2, space="PSUM") as tps, \
                 tc.tile_pool(name="misc_ps", bufs=2, space="PSUM") as mps, \
                 tc.tile_pool(name="fin_sb", bufs=2) as osb:
                to_pend = []

                def oproj(m, toT):
                    o_ps = mps.tile([128, DIM], F32, tag="misc", name="o_ps")
                    for g in range(KC):
                        nc.tensor.matmul(
                            o_ps[:],
                            toT[:, 128 * g:128 * (g + 1)],
                            wo_sb[g][:],
                            start=(g == 0), stop=False)
                    nc.tensor.matmul(o_ps[:], ones_sb[:], bo_sb[:],
                                     start=False, stop=True)
                    o_sb = osb.tile([128, DIM], F32, name="o_sb")
                    if m % 2 == 0:
                        nc.scalar.copy(o_sb[:], o_ps[:])
                    else:
                        nc.vector.tensor_copy(o_sb[:], o_ps[:])
                    nc.sync.dma_start(
                        out=out_ext[128 * m:128 * (m + 1), :], in_=o_sb[:])

                id2_ap = AP(id2_sb.tensor, id2_sb.offset,
                            [[256, 128], [128, 2], [1, 128]])
                NP = MT * (HEADS // 2)   # 32 head pairs, pipelined 3 deep

                def stage_a(k):
                    """T matmuls + evac + tails + skew DMA launch."""
                    m, hp = divmod(k, HEADS // 2)
                    off = 896 - 128 * m
                    lo = max(0, 128 * m - 385)
                    hi = min(1150, 128 * m + 639)
                    w = hi - lo + 1
                    pz = pos8z[k % len(pos8z)]
                    t_pair = asb.tile([128, 2 * TWPAD], FP8, name="t_pair")
                    for hh in range(2):
                        ph = hh * 64
                        qh = qT_sb[hp][ph:ph + 64, 128 * m:128 * (m + 1)]
                        tbase = TWPAD * hh
                        t_chunks = []
                        for ci, (c0, cw) in enumerate(
                                ((lo, 512), (lo + 512, w - 512))):
                            t_ps = tps.tile([128, 512], F32, tag="t_ps",
                                            name=f"t_ps{ci}")
                            t_chunks.append(t_ps)
                            nc.tensor.matmul(
                                t_ps[:, 0:cw],
                                qh,
                                rel_sb[ph:ph + 64, off + c0:off + c0 + cw],
                                start=True, stop=True)
                            if ci == 0:
                                nc.vector.tensor_copy(
                                    t_pair[:, tbase + c0:tbase + c0 + cw],
                                    t_ps[:, 0:cw])
                            else:
                                nc.scalar.copy(
                                    t_pair[:, tbase + c0:tbase + c0 + cw],
                                    t_ps[:, 0:cw])
                        if lo > 0:    # low clip tail: rel_emb[1024] rows
                            nc.vector.tensor_scalar_mul(
                                t_pair[:, tbase:tbase + lo],
                                onesw_sb[:, 0:lo],
                                t_chunks[0][:, 0:1])
                        if hi < 1150:  # high clip tail: rel_emb[0] rows
                            nc.vector.tensor_scalar_mul(
                                t_pair[:, tbase + hi + 1:tbase + 1151],
                                onesw_sb[:, 0:1150 - hi],
                                t_chunks[1][:, w - 513:w - 512])
                    # ONE skew DMA for the pair: pos8z[p, hh, jc, c] =
                    # t_pair[p, TWPAD*hh + 512*jc + c + 127 - p]
                    skew_in = AP(t_pair.tensor, t_pair.offset + 127,
                                 [[2 * TWPAD - 1, 128], [TWPAD, 2],
                                  [512, 2], [1, 512]])
                    skew_out = AP(pz.tensor, pz.offset,
                                  [[2 * PZW, 128], [PZW, 2],
                                   [512, 2], [1, 512]])
                    nc.sync.dma_start(out=skew_out, in_=skew_in)
                    return pz

                def stage_b(k, pz):
                    """S matmuls + DoubleRow pos-add + exp + transpose."""
                    m, hp = divmod(k, HEADS // 2)
                    p_pair = asb.tile([128, 2048], BF16, name="p_pair")
                    for hh in range(2):
                        ph = hh * 64
                        qh = qT_sb[hp][ph:ph + 64, 128 * m:128 * (m + 1)]
                        s_ps = sps.tile([128, 1024], F32, name="s_ps")
                        for jc in range(2):
                            nc.tensor.matmul(
                                s_ps[:, 512 * jc:512 * (jc + 1)],
                                qh,
                                kT_sb[hp][ph:ph + 64, 512 * jc:512 * (jc + 1)],
                                start=True, stop=False)
                        for jc in range(2):
                            rhs = AP(pz.tensor,
                                     pz.offset + PZW * hh + 512 * jc,
                                     [[2 * PZW, 128],
                                      [1024 - 512 * jc, 2], [1, 512]])
                            nc.tensor.matmul(
                                s_ps[:, 512 * jc:512 * (jc + 1)],
                                id2_ap, rhs,
                                start=False, stop=True, perf_mode=DR)
                        nc.scalar.activation(
                            p_pair[:, 1024 * hh:1024 * (hh + 1)],
                            s_ps[:],
                            mybir.ActivationFunctionType.Exp,
                            scale=SCALE)
                    # ONE DMA-transpose: ptT[p, b, c] = p_pair[c, 128b+p]
                    ptT = asb.tile([128, 2048], BF16, name="ptT")
                    ptT_ap = AP(ptT.tensor, ptT.offset,
                                [[2048, 128], [128, 16], [1, 128]])
                    nc.sync.dma_start_transpose(ptT_ap, p_pair[:])
                    return ptT

                def stage_c(k, ptT):
                    """AV matmuls (with z column) + normalize-evacuate."""
                    m, hp = divmod(k, HEADS // 2)
                    for hh in range(2):
                        h = 2 * hp + hh
                        av_ps = mps.tile([128, VW], F32, tag="misc",
                                         name="av_ps")
                        for jb in range(MT):
                            nc.tensor.matmul(
                                av_ps[:],
                                ptT[:, 128 * (8 * hh + jb):
                                    128 * (8 * hh + jb + 1)],
                                v2_sb[jb][:, VW * h:VW * (h + 1)],
                                start=(jb == 0), stop=(jb == MT - 1))
                        r_sb = asb.tile([128, 1], F32, name="r_sb", bufs=4)
                        nc.vector.reciprocal(r_sb[:], av_ps[:, DH:DH + 1])
                        nc.vector.tensor_scalar_mul(
                            o_att[m][:, DH * h:DH * (h + 1)],
                            av_ps[:, 0:DH], r_sb[:])
                    if hp == HEADS // 2 - 1:
                        # last pair of this m: launch o_att transpose + proj
                        toT = asb.tile([128, INNER], BF16, name="toT", bufs=3)
                        to_out = AP(toT.tensor, toT.offset,
                                    [[INNER, 128], [128, KC], [1, 128]])
                        nc.sync.dma_start_transpose(to_out, o_att[m][:])
                        to_pend.append((m, toT))
                        if m > 0:
                            oproj(*to_pend.pop(0))

                pend_a = {}
                pend_b = {}
                DB, DC = 2, 4   # B trails A by 2 pairs, C trails A by 4
                for k in range(NP + DC):
                    if k < NP:
                        pend_a[k] = stage_a(k)
                    if k - DB >= 0 and k - DB < NP:
                        pend_b[k - DB] = stage_b(k - DB, pend_a.pop(k - DB))
                    if k - DC >= 0:
                        stage_c(k - DC, pend_b.pop(k - DC))

                for mm, tt in to_pend:
                    oproj(mm, tt)
    nc.compile()
    return nc


def _prep_inputs(x, Wq, Wkv, rel_emb, Wo, bo):
    import ml_dtypes
    tobf = lambda a: np.asarray(a, dtype=np.float32).astype(ml_dtypes.bfloat16)
    tof8 = lambda a: np.asarray(a, dtype=np.float32).astype(ml_dtypes.float8_e4m3)
    Wk = Wkv[:, :INNER]
    Wv = Wkv[:, INNER:]
    # relT[d, c] = rel_emb[1024 - clip(c - 511, 0, 1024), d], duplicated onto
    # partitions 64..127 so both head-parity quadrants can read it.
    c = np.arange(RELTW)
    rows = RELW - 1 - np.clip(c - (MAX_POS - 1), 0, RELW - 1)
    relT64 = np.ascontiguousarray(rel_emb[rows].T)          # [64, 2047]
    relT = np.concatenate([relT64, relT64], axis=0)         # [128, 2047]
    id2 = np.zeros((128, 256), dtype=np.float32)
    id2[:, :128] = np.eye(128)
    id2[:, 128:] = np.eye(128)
    base = {
        "wq": tobf(Wq), "wk": tobf(Wk), "wv": tobf(Wv), "wo": tobf(Wo),
        "relT": tobf(relT), "bo": tobf(bo.reshape(1, DIM)),
        "id2": tof8(id2),
    }
    in_maps = []
    for c_ in range(B):
        m = dict(base)
        m["xT"] = tobf(np.ascontiguousarray(x[c_].T))
        in_maps.append(m)
    return in_maps


def kernel(x, Wq, Wkv, rel_emb, Wo, bo):
    if "nc" not in _CACHE:
        _CACHE["nc"] = _build_nc()
    nc = _CACHE["nc"]
    in_maps = _prep_inputs(x, Wq, Wkv, rel_emb, Wo, bo)
    res = run_bass_kernel_spmd(nc, in_maps, list(range(B))).results
    out = np.stack([res[c]["out"] for c in range(B)]).astype(np.float32)
    return out
